# revision 11
# baseline (speedup 1.0000x reference)
"""DTW (dynamic time warping) distance kernel for Trainium2, 8-core SPMD.

Problem: B=32 independent (x[b] in R^{1024x64}, y[b] in R^{1024x64}) pairs.
For each pair: dist = cdist(x, y) (euclidean, [1024, 1024]); DTW dynamic
program over dist; output D[N, M] scalar per pair.

Sharding: embarrassingly parallel over batch. 8 cores x 4 batches each.

Per-core algorithm:
  Phase 1 (cdist): dist^2 = xsq_i + ysq_j - 2 x.y^T via one augmented
  matmul per [128, 512] tile (K=65: 64 feature rows of -2*x^T plus a ones
  row pairing with a ysq row); xsq added as the ACT bias of the Relu pass;
  then Sqrt. Tiles are DMAed to one DRAM buffer in 32x32-tile-blocked
  layout dist[b][I][J][r][t] (i = 32I + r, j = 32J + t).

  Phase 2 (DTW): tile-wavefront DP, 2 DVE ops per row-step. The [32, 32]
  tile grid is swept along anti-diagonals w = I + J (63 waves); partition
  p = 32b + I owns tile row I and processes tile (I, w - I) during wave w.
  Within a tile, each of the 32 rows is ONE chunk-free scan:
      X_r[t] = min(Mn_r[t], X_r[t-1]) + c[t],   X_r[-1] = L_r
      Mn_r[t] = min(W_{r-1}[t], W_{r-1}[t+1])   (one tensor_tensor)
  where W packs [L_r, X_r[0..31]] per row at pitch 33, so the left
  boundary L_r (right column of the west tile, same partition, previous
  wave) enters as the scan's per-partition initial, and the top boundary
  (bottom row + corner of the north tile) moves down one partition via a
  single stream_shuffle of the previous wave's last W row slot. Inactive
  partitions stay at BIG automatically: their W starts BIG and
  min(BIG, BIG + c) = BIG for any cost c >= 0 (ring memset to 0).
"""

import numpy as np

import concourse.bass as bass
import concourse.bacc as bacc
import concourse.mybir as mybir
from concourse.tile import TileContext
from concourse.masks import make_identity
from concourse import bass_utils

f32 = mybir.dt.float32
ADD = mybir.AluOpType.add
MIN = mybir.AluOpType.min
MAX = mybir.AluOpType.max
MULT = mybir.AluOpType.mult
ACT = mybir.ActivationFunctionType

N_CORES = 8
NB = 4          # batches per core
N = 1024        # rows (x length)
M = 1024        # cols (y length)
F = 64          # features
T = 32          # DP tile edge
G = 32          # tile grid edge (G*T == N == M)
NW = 2 * G - 1  # waves
BIG = 3.0e38    # finite stand-in for +inf
SHIFT1 = [0] + list(range(31))  # stream_shuffle: out[m] = in[m-1] per 32-block

BSZ = G * G * T * T       # dist elements per batch (1 Mi)
ISZ = G * T * T           # dist elements per tile row I (32 Ki)


def _emit_cdist(nc, sb, ps, psmm, x_d, y_d, dist_d, n_rows):
    """Emit phase 1. dist_d: DRAM [NB * BSZ], tile-blocked layout."""
    n_ti = n_rows // 128

    ident = sb.tile([128, 128], f32)
    make_identity(nc, ident[:])
    ones64 = sb.tile([64, 1], f32)
    nc.vector.memset(ones64[:], 1.0)

    XTA, YTA, XSQ = [], [], []
    for b in range(NB):
        XN = sb.tile([128, 8 * F], f32, tag="XN", bufs=2)
        YN = sb.tile([128, 8 * F], f32, tag="YN", bufs=2)
        xta = sb.tile([65, N], f32, tag=f"XTA{b}")
        yta = sb.tile([65, M], f32, tag=f"YTA{b}")
        xsq = sb.tile([128, 8], f32, tag=f"XSQ{b}")
        ysqel = sb.tile([64, M], f32, tag="YSQel", bufs=2)
        sqs = sb.tile([128, F], f32, tag="sqs", bufs=2)

        # natural-layout loads: partition = i%128, free = (i//128, f).
        nc.gpsimd.dma_start(
            XN[:], bass.AP(x_d, b * N * F, [[F, 128], [128 * F, 8], [1, F]])
        )
        nc.gpsimd.dma_start(
            YN[:], bass.AP(y_d, b * M * F, [[F, 128], [128 * F, 8], [1, F]])
        )

        # PE transposes -> feature-major; x scaled by -2 on the PSUM copy-out.
        for g in range(2):
            pt = ps.tile([64, 512], f32, tag="pt")
            for tt in range(4):
                t = 4 * g + tt
                nc.tensor.transpose(
                    pt[:, tt * 128 : (tt + 1) * 128],
                    YN[:, t * F : (t + 1) * F], ident[:],
                )
            nc.scalar.activation(yta[0:64, g * 512 : (g + 1) * 512], pt[:], ACT.Copy)
        for g in range(max(1, n_ti // 4)):
            pt = ps.tile([64, 512], f32, tag="pt")
            nt = min(4, n_ti - 4 * g)
            for tt in range(nt):
                t = 4 * g + tt
                nc.tensor.transpose(
                    pt[:, tt * 128 : (tt + 1) * 128],
                    XN[:, t * F : (t + 1) * F], ident[:],
                )
            nc.scalar.activation(
                xta[0:64, g * 512 : g * 512 + nt * 128],
                pt[:, 0 : nt * 128], ACT.Copy, scale=-2.0,
            )
        # xsq[i] per i-tile column (ACT Square with accumulate)
        for t in range(n_ti):
            nc.scalar.activation(
                sqs[:], XN[:, t * F : (t + 1) * F], ACT.Square,
                accum_out=xsq[:, t : t + 1],
            )
        # augmented rows: xta row 64 = ones; yta row 64 = ysq
        nc.vector.memset(xta[64:65, :], 1.0)
        nc.gpsimd.tensor_tensor(ysqel[:], yta[0:64, :], yta[0:64, :], MULT)
        for nj in range(2):
            py = ps.tile([1, 512], f32, tag="py")
            nc.tensor.matmul(
                py[:], ones64[:], ysqel[:, nj * 512 : (nj + 1) * 512],
                start=True, stop=True,
            )
            nc.scalar.activation(
                yta[64:65, nj * 512 : (nj + 1) * 512], py[:], ACT.Copy
            )
        XTA.append(xta)
        YTA.append(yta)
        XSQ.append(xsq)

    # dist tiles: matmul + relu(+xsq bias) + sqrt + DMA out tile-blocked.
    for ti in range(n_ti):
        for b in range(NB):
            ds2 = sb.tile([128, 1024], f32, tag="DS2", bufs=2)
            for nj in range(2):
                pq = psmm.tile([128, 512], f32, tag="pq")
                nc.tensor.matmul(
                    pq[:],
                    XTA[b][:, ti * 128 : (ti + 1) * 128],
                    YTA[b][:, nj * 512 : (nj + 1) * 512],
                    start=True, stop=True,
                )
                ds = sb.tile([128, 512], f32, tag="DS", bufs=3)
                nc.scalar.activation(
                    ds[:], pq[:], ACT.Relu, bias=XSQ[b][:, ti : ti + 1]
                )
                nc.scalar.activation(
                    ds2[:, nj * 512 : (nj + 1) * 512], ds[:], ACT.Sqrt
                )
            # -> dist_d[b][I][J][r][t] with I = 4 ti + Ii, i_local = 32 Ii + r
            for Ii in range(4):
                dst = bass.AP(
                    dist_d, b * BSZ + (ti * 4 + Ii) * ISZ,
                    [[T, T], [T * T, G], [1, T]],
                )
                src = bass.AP(
                    ds2.tensor, Ii * 32 * 1024,
                    [[1024, T], [T, G], [1, T]],
                )
                nc.sync.dma_start(dst, src)


def _emit_dtw(nc, sb, dist_d, xout_d):
    """Emit phase 2: skewed-slot wavefront, two slots interleaved.

    Tile (I, J) runs in slot s = 2I + J (94 slots); a new slot starts
    every 16 row-steps so exactly two slots are in flight, and their ops
    are interleaved so every DVE op's producer is >= 2 instructions back
    (hides the DVE inter-instruction RAW bubble, ~233 -> ~120 ns/op).

    Storage: two parity buffers, row pitch 66. Slot s (parity p) keeps
    its W rows [L_r, X_r[0..31]] at buf[p][:, r*66 : r*66+33]; its Mn
    lives in the OTHER buffer at buf[1-p][:, r*66+33 : r*66+65], directly
    after col r*66+32 which is slot s-1's X_r[31] = this slot's left
    boundary. The X scan (data0 = buf[1-p][:, r*66+32 : +65], data1 =
    [0, c_r] ring row, initial BIG) then computes
        out[0] = L_r,  out[1+t] = X_r[t]
    in one instruction with no per-partition initial and no L copies.
    The top boundary [corner, bottom row] comes from the same-parity
    buffer (slot s-2) row 31 via one stream_shuffle per slot.
    """
    NSLOT = 4
    RP = 33 * T                         # ring slot pitch
    P2 = 66                             # W row pitch
    NS = 3 * G - 2                      # slots: s = 2I + J in [0, 93]
    LAG = 16                            # row-steps between slot starts
    ring = sb.tile([128, NSLOT * RP], f32)
    buf = [
        sb.tile([128, P2 * T], f32, tag=f"W{k}", name=f"skbuf{k}")
        for k in range(2)
    ]
    TOPr = [
        sb.tile([128, 33], f32, name=f"TOPr{k}") for k in range(2)
    ]
    TOPf = [
        sb.tile([128, 33], f32, name=f"TOPf{k}") for k in range(2)
    ]
    INJ = sb.tile([128, 1], f32)

    ring_pitch = NSLOT * RP

    nc.vector.memset(ring[:], 0.0)      # inactive lanes see costs >= 0
    for k in range(2):
        nc.vector.memset(buf[k][:], BIG)
    nc.vector.memset(INJ[:], -BIG)
    for b in range(NB):                 # I = 0 lanes: top boundary is BIG
        nc.vector.memset(INJ[32 * b : 32 * b + 1, :], BIG)

    def start_slot(s):
        p = s % 2
        slot = (s % NSLOT) * RP
        # diagonal load: partition 32b + I gets tile (I, s - 2I); row r
        # lands at ring[:, slot + r*33 + 1 : +33] (col r*33 stays 0).
        ilo = max(0, (s - (G - 1) + 1) // 2)
        ihi = min(G - 1, s // 2)
        cnt = ihi - ilo + 1
        for b in range(NB):
            dst = bass.AP(
                ring.tensor, (32 * b + ilo) * ring_pitch + slot + 1,
                [[ring_pitch, cnt], [33, T], [1, T]],
            )
            src = bass.AP(
                dist_d, b * BSZ + ilo * ISZ + (s - 2 * ilo) * T * T,
                [[ISZ - 2 * T * T, cnt], [T, T], [1, T]],
            )
            nc.gpsimd.dma_start(dst, src)
        # top boundary from slot s-2 (same parity) row 31, one partition
        # down; TOPr[32b] garbage -> forced BIG via INJ.
        nc.vector.stream_shuffle(
            TOPr[p][:], buf[p][:, 31 * P2 : 31 * P2 + 33], SHIFT1
        )
        nc.vector.scalar_tensor_tensor(
            TOPf[p][:], TOPr[p][:], INJ[:, 0:1], TOPr[p][:], MAX, MAX
        )
        if s == 0:
            for b in range(NB):         # D[0,0] corner
                nc.vector.memset(TOPf[p][32 * b : 32 * b + 1, 0:1], 0.0)

    def row_tt(s, r):
        p = s % 2
        if r == 0:
            lo, hi = TOPf[p][:, 0:T], TOPf[p][:, 1 : T + 1]
        else:
            base = (r - 1) * P2
            lo = buf[p][:, base : base + T]
            hi = buf[p][:, base + 1 : base + T + 1]
        nc.vector.tensor_tensor(
            buf[1 - p][:, r * P2 + 33 : r * P2 + 65], lo, hi, MIN
        )

    def row_scan(s, r):
        p = s % 2
        slot = (s % NSLOT) * RP
        nc.vector.tensor_tensor_scan(
            buf[p][:, r * P2 : r * P2 + 33],
            buf[1 - p][:, r * P2 + 32 : r * P2 + 65],
            ring[:, slot + r * 33 : slot + r * 33 + 33],
            BIG, MIN, ADD,
        )

    active = []                         # (slot, next_row)
    t = 0
    done = 0
    while done < NS:
        if t % LAG == 0 and t // LAG < NS:
            s = t // LAG
            start_slot(s)
            active.append([s, 0])
        for a in active:
            row_tt(a[0], a[1])
        for a in active:
            row_scan(a[0], a[1])
            a[1] += 1
        done += sum(1 for a in active if a[1] == T)
        active = [a for a in active if a[1] < T]
        t += 1

    nc.sync.dma_start(
        xout_d[:], buf[(NS - 1) % 2][:, 31 * P2 + 32 : 31 * P2 + 33]
    )


def build_nc(n_rows=N):
    nc = bacc.Bacc()
    x_d = nc.dram_tensor("x", [NB, N, F], f32, kind="ExternalInput")
    y_d = nc.dram_tensor("y", [NB, M, F], f32, kind="ExternalInput")
    xout_d = nc.dram_tensor("xout", [128, 1], f32, kind="ExternalOutput")

    with TileContext(nc) as tc:
        with (
            tc.tile_pool(name="sb", bufs=1) as sb,
            tc.tile_pool(name="ps", bufs=2, space="PSUM") as ps,
            tc.tile_pool(name="psmm", bufs=4, space="PSUM") as psmm,
            tc.tile_pool(name="dr", bufs=1, space="DRAM") as dr,
        ):
            dist_t = dr.tile([NB * BSZ], f32, name="distbuf")
            _emit_cdist(nc, sb, ps, psmm, x_d, y_d, dist_t.tensor, n_rows)
            _emit_dtw(nc, sb, dist_t.tensor, xout_d)
    nc.compile()
    return nc


_NC_CACHE = {}


def _get_nc(n_rows=N):
    if n_rows not in _NC_CACHE:
        _NC_CACHE[n_rows] = build_nc(n_rows)
    return _NC_CACHE[n_rows]


def _make_in_maps(x, y):
    return [
        {"x": np.ascontiguousarray(x[NB * c : NB * (c + 1)]),
         "y": np.ascontiguousarray(y[NB * c : NB * (c + 1)])}
        for c in range(N_CORES)
    ]


def _extract_out(results):
    out = np.empty((N_CORES * NB,), np.float32)
    for c in range(N_CORES):
        xo = results[c]["xout"]
        for b in range(NB):
            out[NB * c + b] = xo[32 * b + 31, 0]
    return out


def kernel(x: np.ndarray, y: np.ndarray) -> np.ndarray:
    """x, y: [32, 1024, 64] float32 -> [32] float32 of DTW distances."""
    x = np.ascontiguousarray(x, dtype=np.float32)
    y = np.ascontiguousarray(y, dtype=np.float32)
    nc = _get_nc()
    res = bass_utils.run_bass_kernel_spmd(
        nc, _make_in_maps(x, y), core_ids=list(range(N_CORES))
    )
    return _extract_out(res.results)
